# revision 1
# baseline (speedup 1.0000x reference)
"""Trainium2 Bass kernel for nn_DistanceFieldPenetrationLoss.

Computes loss = sum(relu(1e-3 - tridist(A,B))) / count over 2M close pairs,
sharded data-parallel over 8 NeuronCores. Per-pair triangle rows are
pre-gathered on the host (HW indirect-DMA gathers are one-index-per-
partition on TRN2, making on-device gathers descriptor-bound) and streamed
to SBUF as contiguous DMA; all geometry runs on-device in a term-blocked
SoA layout (strided/broadcast AP views over the gathered tile, 1 instr per
blocked group of the 15 distance terms).

The per-pair triangle "distance" replicates the reference exactly:
min over 15 terms: 6 point-(column-)triangle distances + 9 row-edge/edge
distances. Point-triangle = {face-masked, 3 point-edge}; edge-edge is
evaluated in its exact boundary form: min(interior-masked, 4 point-edge),
which equals the reference's clamp/recompute algorithm mathematically.
"""
import numpy as np

import concourse.bass as bass
import concourse.bacc as bacc
import concourse.mybir as mybir
import concourse.tile as tile
from concourse.bass_utils import run_bass_kernel_spmd

F32 = mybir.dt.float32
I32 = mybir.dt.int32
Alu = mybir.AluOpType
Act = mybir.ActivationFunctionType

P = 128
B, F, PPB = 4, 50000, 500000
NPAIR = B * PPB
NCORE = 8
PER_CORE = NPAIR // NCORE          # 250000
NCOL = 1954                        # 128*1954 = 250112 slots per core
CAP = P * NCOL
import os
if os.environ.get("K_DEBUG_SMALL"):
    NCOL = 8
    CAP = P * NCOL
    TILE_W = [8]
else:
    TILE_W = [152] * 12 + [130]        # sum = 1954
EPS = 1e-12
LOSS_EPS = 1e-3
BIG = 1e30

_CACHE = {}


def _mk(ap, off, dims):
    """View into an SBUF tile AP with explicit free dims [[step, count], ...]."""
    return bass.AP(ap.tensor, ap.offset + off, [list(ap.ap[0])] + [list(d) for d in dims])


def _build_kernel(ncol=None, tile_w=None):
    NCOL = ncol if ncol is not None else globals()["NCOL"]
    TILE_W = tile_w if tile_w is not None else globals()["TILE_W"]
    nc = bacc.Bacc("TRN2", target_bir_lowering=False, debug=False)
    gdata = nc.declare_dram_parameter("gdata", [P, 18 * NCOL], F32, isOutput=False)
    maskin = nc.declare_dram_parameter("maskin", [P, NCOL], F32, isOutput=False)
    psum_out = nc.declare_dram_parameter("psum", [P, len(TILE_W)], F32, isOutput=True)
    DEBUG = bool(os.environ.get("K_DEBUG_SMALL"))
    if DEBUG:
        dist_out = nc.declare_dram_parameter("dist", [P, NCOL], F32, isOutput=True)
        accpt_out = nc.declare_dram_parameter("accpt", [P, NCOL], F32, isOutput=True)

    with tile.TileContext(nc) as tc:
        with (
            tc.tile_pool(name="gio", bufs=2) as gio,
            tc.tile_pool(name="work", bufs=1) as work,
        ):
            V = nc.vector
            S = nc.scalar

            psum_t = work.tile([P, len(TILE_W)], F32, tag="psum", name="psum")
            V.memset(psum_t[:], 0.0)

            colbase = 0
            for ti, W in enumerate(TILE_W):
                G = gio.tile([P, 18 * W], F32, tag="g", name="g")
                nc.sync.dma_start(out=G[:], in_=gdata[:, 18 * colbase:18 * (colbase + W)])
                M = gio.tile([P, W], F32, tag="mask", name="mask")
                nc.sync.dma_start(out=M[:], in_=maskin[:, colbase:colbase + W])

                Gap = G[:]

                # --- G views.  free index = w*18 + t*9 + m;  m = 3*row + col(coord)
                # row-vertex i, comp c of side t:   m = 3i + c
                # col-vertex k, comp c of side t:   m = 3c + k
                def Rblk(t, c):          # [3(vert i), W]
                    return _mk(Gap, 9 * t + c, [[3, 3], [18, W]])

                def R9A(c):              # [3(i), 3(rep j), W]
                    return _mk(Gap, c, [[3, 3], [0, 3], [18, W]])

                def R9B(c):              # [3(rep i), 3(j), W]
                    return _mk(Gap, 9 + c, [[0, 3], [3, 3], [18, W]])

                def Cv(t, c, k):         # [W] single col-vertex comp
                    return _mk(Gap, 9 * t + 3 * c + k, [[18, W]])

                def Cb3(t, c, k):        # [3(rep), W]
                    return _mk(Gap, 9 * t + 3 * c + k, [[0, 3], [18, W]])

                # tile allocation helpers (plain + shaped views)
                def TW(tag):
                    return work.tile([P, W], F32, tag=tag, name=tag)[:]

                def T3(tag):
                    return work.tile([P, 3 * W], F32, tag=tag, name=tag)[:]

                def T9(tag):
                    return work.tile([P, 9 * W], F32, tag=tag, name=tag)[:]

                def s3(ap):              # [3, W] view of 3W tile
                    return _mk(ap, 0, [[W, 3], [1, W]])

                def s9(ap):              # [3, 3, W] view of 9W tile
                    return _mk(ap, 0, [[3 * W, 3], [W, 3], [1, W]])

                def b3(ap_w):            # broadcast [W] tile over 3 blocks
                    return _mk(ap_w, 0, [[0, 3], [1, W]])

                def repA(ap3):           # [3W] tile -> [3(i), 3(rep), W]
                    return _mk(ap3, 0, [[W, 3], [0, 3], [1, W]])

                def repB(ap3):           # [3W] tile -> [3(rep), 3(j), W]
                    return _mk(ap3, 0, [[0, 3], [W, 3], [1, W]])

                def blkof(ap3, i):       # i-th W block of a 3W tile
                    return _mk(ap3, i * W, [[1, W]])

                GP = nc.gpsimd

                def tt(out, a, b, op, eng=None):
                    (eng or V).tensor_tensor(out=out, in0=a, in1=b, op=op)

                def dot3g(out, av, bv, tmp):
                    GP.tensor_tensor(out=tmp, in0=av[0], in1=bv[0], op=Alu.mult)
                    GP.tensor_tensor(out=out, in0=av[1], in1=bv[1], op=Alu.mult)
                    GP.tensor_tensor(out=out, in0=out, in1=tmp, op=Alu.add)
                    GP.tensor_tensor(out=tmp, in0=av[2], in1=bv[2], op=Alu.mult)
                    GP.tensor_tensor(out=out, in0=out, in1=tmp, op=Alu.add)

                def dot3(out, av, bv, tmp):
                    tt(tmp, av[0], bv[0], Alu.mult)
                    tt(out, av[1], bv[1], Alu.mult)
                    tt(out, out, tmp, Alu.add)
                    tt(tmp, av[2], bv[2], Alu.mult)
                    tt(out, out, tmp, Alu.add)

                def norm2(out, av, tmp):
                    S.activation(out=tmp, in_=av[0], func=Act.Square)
                    S.activation(out=out, in_=av[1], func=Act.Square)
                    tt(out, out, tmp, Alu.add)
                    S.activation(out=tmp, in_=av[2], func=Act.Square)
                    tt(out, out, tmp, Alu.add)

                def recip(out, x, tmp):
                    S.activation(out=tmp, in_=x, func=Act.Ln)
                    S.activation(out=out, in_=tmp, func=Act.Exp, scale=-1.0)

                def clip01(x):
                    V.tensor_scalar(out=x, in0=x, scalar1=0.0, scalar2=1.0,
                                    op0=Alu.max, op1=Alu.min)

                acc = TW("acc")
                V.memset(acc, BIG)

                def foldmin(blocked_ap_tile, nblk):
                    # min of nblk W-blocks of a tile into acc
                    n = nblk
                    while n > 1:
                        h = n // 2
                        lo = _mk(blocked_ap_tile, 0, [[1, h * W]])
                        hi = _mk(blocked_ap_tile, (n - h) * W, [[1, h * W]])
                        tt(lo, lo, hi, Alu.min)
                        n = n - h
                    tt(acc, acc, _mk(blocked_ap_tile, 0, [[1, W]]), Alu.min)

                # ---------- per-side derived data ----------
                side = []
                for t in (0, 1):
                    sd = {}
                    # column-triangle data
                    eC = []
                    for pair_kk in ((1, 0), (2, 0), (2, 1)):
                        comps = []
                        for c in range(3):
                            e = TW(f"eC{len(eC)}{c}_{t}")
                            tt(e, Cv(t, c, pair_kk[0]), Cv(t, c, pair_kk[1]), Alu.subtract)
                            comps.append(e)
                        eC.append(comps)
                    sd["eC"] = eC
                    tmpw = TW(f"tmpw{t}")
                    aC = TW(f"aC{t}"); norm2(aC, eC[0], tmpw)
                    V.tensor_scalar(out=aC, in0=aC, scalar1=EPS, scalar2=None, op0=Alu.max)
                    bC = TW(f"bC{t}"); dot3(bC, eC[0], eC[1], tmpw)
                    cC = TW(f"cC{t}"); norm2(cC, eC[1], tmpw)
                    V.tensor_scalar(out=cC, in0=cC, scalar1=EPS, scalar2=None, op0=Alu.max)
                    a2C = TW(f"a2C{t}"); norm2(a2C, eC[2], tmpw)
                    V.tensor_scalar(out=a2C, in0=a2C, scalar1=EPS, scalar2=None, op0=Alu.max)
                    det = TW(f"det{t}")
                    S.activation(out=tmpw, in_=bC, func=Act.Square)
                    tt(det, aC, cC, Alu.mult)
                    tt(det, det, tmpw, Alu.subtract)
                    V.tensor_scalar(out=det, in0=det, scalar1=EPS, scalar2=None, op0=Alu.max)
                    for nm, src in (("invdet", det), ("invaC", aC), ("invcC", cC), ("inva2C", a2C)):
                        dst = TW(nm + str(t)); recip(dst, src, tmpw)
                        sd[nm] = dst
                    sd.update(aC=aC, bC=bC, cC=cC, a2C=a2C, det=det)

                    # row-edge data (3W blocked: edges [R1-R0, R2-R1, R0-R2])
                    E = []
                    for c in range(3):
                        e = T3(f"E{c}_{t}")
                        # blocks 0..1: R_{i+1} - R_i
                        nxt = _mk(Gap, 9 * t + c + 3, [[3, 2], [18, W]])
                        cur = _mk(Gap, 9 * t + c, [[3, 2], [18, W]])
                        tt(_mk(e, 0, [[W, 2], [1, W]]), nxt, cur, Alu.subtract)
                        # block 2: R0 - R2
                        tt(blkof(e, 2), _mk(Gap, 9 * t + c, [[18, W]]),
                           _mk(Gap, 9 * t + c + 6, [[18, W]]), Alu.subtract)
                        E.append(e)
                    sd["E"] = E
                    tmp3 = T3(f"tmp3_{t}")
                    aE = T3(f"aE{t}")
                    norm2(aE, E, tmp3)
                    invE = T3(f"invE{t}"); recip(invE, aE, tmp3)
                    ninvE = T3(f"ninvE{t}")
                    V.tensor_scalar(out=ninvE, in0=invE, scalar1=-1.0, scalar2=None, op0=Alu.mult)
                    sd.update(aE=aE, invE=invE, ninvE=ninvE)
                    side.append(sd)

                # ---------- point-triangle, 2 directions x 3 points (3W blocked) ----------
                t3a = T3("t3a"); t3b = T3("t3b"); t3c = T3("t3c")
                for tp, tt_ in ((0, 1), (1, 0)):
                    sd = side[tt_]
                    w = [T3(f"w{c}") for c in range(3)]
                    for c in range(3):
                        tt(s3(w[c]), Rblk(tp, c), Cb3(tt_, c, 0), Alu.subtract)
                    d = T3("ptd"); dot3(d, [b3(x) for x in sd["eC"][0]], w, t3a)
                    e = T3("pte"); dot3(e, [b3(x) for x in sd["eC"][1]], w, t3a)
                    f = T3("ptf"); norm2(f, w, t3a)
                    s = T3("pts")
                    tt(t3a, b3(sd["bC"]), e, Alu.mult)
                    tt(t3b, b3(sd["cC"]), d, Alu.mult)
                    tt(s, t3a, t3b, Alu.subtract)
                    t = T3("ptt")
                    tt(t3a, b3(sd["bC"]), d, Alu.mult)
                    tt(t3b, b3(sd["aC"]), e, Alu.mult)
                    tt(t, t3a, t3b, Alu.subtract)
                    # in-face margin m = min(s, t, det-(s+t))
                    m = T3("ptm")
                    tt(m, s, t, Alu.min)
                    tt(t3a, s, t, Alu.add)
                    V.scalar_tensor_tensor(out=t3b, in0=t3a, scalar=-1.0, in1=b3(sd["det"]),
                                           op0=Alu.mult, op1=Alu.add)
                    tt(m, m, t3b, Alu.min)
                    # face distance
                    fc = T3("ptfc")
                    tt(t3a, d, s, Alu.mult)
                    tt(t3b, e, t, Alu.mult)
                    tt(t3a, t3a, t3b, Alu.add)
                    tt(t3b, f, b3(sd["det"]), Alu.mult)
                    tt(t3a, t3b, t3a, Alu.subtract)
                    tt(fc, t3a, b3(sd["invdet"]), Alu.mult)
                    V.tensor_scalar(out=fc, in0=fc, scalar1=0.0, scalar2=None, op0=Alu.max)
                    V.tensor_scalar(out=t3a, in0=m, scalar1=0.0, scalar2=BIG,
                                    op0=Alu.is_lt, op1=Alu.mult)
                    tt(fc, fc, t3a, Alu.add)
                    foldmin(fc, 3)
                    # pe01 / pe02: foot on eC0 (param d/aC) and eC1 (param e/cC)
                    for dotv, inv, ev in ((d, "invaC", 0), (e, "invcC", 1)):
                        u = t3c
                        tt(u, dotv, b3(sd[inv]), Alu.mult)
                        clip01(u)
                        pe = T3("ptpe")
                        for c in range(3):
                            tt(t3a, u, b3(sd["eC"][ev][c]), Alu.mult)
                            tt(w2c := t3b, w[c], t3a, Alu.subtract)
                            if c == 0:
                                S.activation(out=pe, in_=w2c, func=Act.Square)
                            else:
                                S.activation(out=t3a, in_=w2c, func=Act.Square)
                                tt(pe, pe, t3a, Alu.add)
                        foldmin(pe, 3)
                    # pe12: w2 = p - C1, edge eC2, param dd/a2C
                    w2 = [T3(f"w2{c}") for c in range(3)]
                    for c in range(3):
                        tt(s3(w2[c]), Rblk(tp, c), Cb3(tt_, c, 1), Alu.subtract)
                    dd = T3("ptdd"); dot3(dd, [b3(x) for x in sd["eC"][2]], w2, t3a)
                    u = t3c
                    tt(u, dd, b3(sd["inva2C"]), Alu.mult)
                    clip01(u)
                    pe = T3("ptpe")
                    for c in range(3):
                        tt(t3a, u, b3(sd["eC"][2][c]), Alu.mult)
                        tt(t3b, w2[c], t3a, Alu.subtract)
                        if c == 0:
                            S.activation(out=pe, in_=t3b, func=Act.Square)
                        else:
                            S.activation(out=t3a, in_=t3b, func=Act.Square)
                            tt(pe, pe, t3a, Alu.add)
                    foldmin(pe, 3)

                if DEBUG:
                    nc.sync.dma_start(out=accpt_out[:, colbase:colbase + W], in_=acc)
                # ---------- edge-edge, 9-blocked [3(i:A-edge), 3(j:B-edge), W] ----------
                EA, EB = side[0]["E"], side[1]["E"]
                t9a = T9("t9a"); t9b = T9("t9b"); t9c = T9("t9c")
                r = [T9(f"r{c}") for c in range(3)]
                for c in range(3):
                    tt(s9(r[c]), R9A(c), R9B(c), Alu.subtract)
                d1v = [repA(EA[c]) for c in range(3)]
                d2v = [repB(EB[c]) for c in range(3)]
                cdot = T9("cdot"); dot3(cdot, d1v, r, t9a)
                fdot = T9("fdot"); dot3(fdot, d2v, r, t9a)
                bq = T9("bq"); dot3(bq, d1v, d2v, t9a)
                aA_r = repA(side[0]["aE"]); aB_t = repB(side[1]["aE"])
                den = T9("den")
                tt(den, aA_r, aB_t, Alu.mult)
                S.activation(out=t9a, in_=bq, func=Act.Square)
                tt(den, den, t9a, Alu.subtract)
                dadj = T9("dadj")
                V.tensor_scalar(out=dadj, in0=den, scalar1=EPS, scalar2=None, op0=Alu.max)
                invd = T9("invd")
                S.activation(out=t9a, in_=dadj, func=Act.Ln)
                S.activation(out=invd, in_=t9a, func=Act.Exp, scale=-1.0)
                s = T9("ees")
                tt(t9a, bq, fdot, Alu.mult)
                tt(t9b, cdot, aB_t, Alu.mult)
                tt(s, t9a, t9b, Alu.subtract)
                tt(s, s, invd, Alu.mult)
                t = T9("eet")
                tt(t9a, aA_r, fdot, Alu.mult)
                tt(t9b, bq, cdot, Alu.mult)
                tt(t, t9a, t9b, Alu.subtract)
                tt(t, t, invd, Alu.mult)
                # Clamp s,t to [0,1]: identical when the interior solution is
                # valid; otherwise the clamped point-pair distance upper-bounds
                # the true segment distance, which the boundary point-edge
                # terms below already realize — the 15-term min is unchanged.
                clip01(s)
                clip01(t)
                d2i = T9("d2i")
                for c in range(3):
                    tt(t9a, s, d1v[c], Alu.mult)
                    tt(t9a, r[c], t9a, Alu.add)
                    tt(t9b, t, d2v[c], Alu.mult)
                    tt(t9a, t9a, t9b, Alu.subtract)
                    if c == 0:
                        S.activation(out=d2i, in_=t9a, func=Act.Square)
                    else:
                        S.activation(out=t9b, in_=t9a, func=Act.Square)
                        tt(d2i, d2i, t9b, Alu.add)
                foldmin(d2i, 9)
                # pe(A_i, B-edge j): u = clip(fdot * invE_B); v = r - u*d2
                u = t9c
                tt(u, fdot, repB(side[1]["invE"]), Alu.mult)
                clip01(u)
                pe9 = T9("pe9")
                for c in range(3):
                    tt(t9a, u, d2v[c], Alu.mult)
                    tt(t9a, r[c], t9a, Alu.subtract)
                    if c == 0:
                        S.activation(out=pe9, in_=t9a, func=Act.Square)
                    else:
                        S.activation(out=t9b, in_=t9a, func=Act.Square)
                        tt(pe9, pe9, t9b, Alu.add)
                foldmin(pe9, 9)
                # pe(B_j, A-edge i): u = clip(cdot * -invE_A); v = r + u*d1
                tt(u, cdot, repA(side[0]["ninvE"]), Alu.mult)
                clip01(u)
                for c in range(3):
                    tt(t9a, u, d1v[c], Alu.mult)
                    tt(t9a, r[c], t9a, Alu.add)
                    if c == 0:
                        S.activation(out=pe9, in_=t9a, func=Act.Square)
                    else:
                        S.activation(out=t9b, in_=t9a, func=Act.Square)
                        tt(pe9, pe9, t9b, Alu.add)
                foldmin(pe9, 9)

                # ---------- pen = relu(1e-3 - sqrt(acc)); masked accumulate ----------
                dist = TW("dist")
                S.activation(out=dist, in_=acc, func=Act.Sqrt)
                pen = TW("pen")
                V.tensor_scalar(out=pen, in0=dist, scalar1=-1.0, scalar2=LOSS_EPS,
                                op0=Alu.mult, op1=Alu.add)
                if DEBUG:
                    nc.sync.dma_start(out=dist_out[:, colbase:colbase + W], in_=dist)
                penm = TW("penm")
                V.scalar_tensor_tensor(out=penm, in0=pen, scalar=0.0, in1=M[:],
                                       op0=Alu.max, op1=Alu.mult,
                                       accum_out=psum_t[:, ti:ti + 1])
                colbase += W

            nc.sync.dma_start(out=psum_out[:], in_=psum_t[:])
    nc.compile()
    return nc




M2 = 4e-6        # (2e-3)^2 certified prune margin squared
N2MIN = 1e-4     # min |cross|^2 for a trustworthy normal direction

NCOL1 = 1954
TILE_W1 = [128] * 15 + [34]
CAP2_COL = 128                     # phase-2 slots per core = 128*128 = 16384
CAP2 = P * CAP2_COL


def _build_flags():
    """Phase-1: certified lower-bound prune. Writes per-slot flag:
    1.0 = certifiably all 15 terms > 1e-3 (pen == 0), 0.0 = needs phase 2."""
    NCOL = NCOL1
    TILE_W = TILE_W1
    nc = bacc.Bacc("TRN2", target_bir_lowering=False, debug=False)
    gdata = nc.declare_dram_parameter("gdata", [P, 18 * NCOL], F32, isOutput=False)
    flags_out = nc.declare_dram_parameter("flags", [P, NCOL], F32, isOutput=True)

    with tile.TileContext(nc) as tc:
        with (
            tc.tile_pool(name="gio", bufs=2) as gio,
            tc.tile_pool(name="work", bufs=1) as work,
        ):
            V = nc.vector
            S = nc.scalar
            colbase = 0
            for ti, W in enumerate(TILE_W):
                G = gio.tile([P, 18 * W], F32, tag="g", name="g")
                nc.sync.dma_start(out=G[:], in_=gdata[:, 18 * colbase:18 * (colbase + W)])
                Gap = G[:]

                def Rblk(t, c):
                    return _mk(Gap, 9 * t + c, [[3, 3], [18, W]])

                def R9A(c):
                    return _mk(Gap, c, [[3, 3], [0, 3], [18, W]])

                def R9B(c):
                    return _mk(Gap, 9 + c, [[0, 3], [3, 3], [18, W]])

                def Cv(t, c, k):
                    return _mk(Gap, 9 * t + 3 * c + k, [[18, W]])

                def Cb3(t, c, k):
                    return _mk(Gap, 9 * t + 3 * c + k, [[0, 3], [18, W]])

                def TW(tag):
                    return work.tile([P, W], F32, tag=tag, name=tag)[:]

                def T3(tag):
                    return work.tile([P, 3 * W], F32, tag=tag, name=tag)[:]

                def T9(tag):
                    return work.tile([P, 9 * W], F32, tag=tag, name=tag)[:]

                def s3(ap):
                    return _mk(ap, 0, [[W, 3], [1, W]])

                def s9(ap):
                    return _mk(ap, 0, [[3 * W, 3], [W, 3], [1, W]])

                def b3(ap_w):
                    return _mk(ap_w, 0, [[0, 3], [1, W]])

                def repA(ap3):
                    return _mk(ap3, 0, [[W, 3], [0, 3], [1, W]])

                def repB(ap3):
                    return _mk(ap3, 0, [[0, 3], [W, 3], [1, W]])

                def blkof(ap3, i):
                    return _mk(ap3, i * W, [[1, W]])

                def tt(out, a, b, op):
                    V.tensor_tensor(out=out, in0=a, in1=b, op=op)

                def dot3(out, av, bv, tmp):
                    tt(tmp, av[0], bv[0], Alu.mult)
                    tt(out, av[1], bv[1], Alu.mult)
                    tt(out, out, tmp, Alu.add)
                    tt(tmp, av[2], bv[2], Alu.mult)
                    tt(out, out, tmp, Alu.add)

                def norm2(out, av, tmp):
                    S.activation(out=tmp, in_=av[0], func=Act.Square)
                    S.activation(out=out, in_=av[1], func=Act.Square)
                    tt(out, out, tmp, Alu.add)
                    S.activation(out=tmp, in_=av[2], func=Act.Square)
                    tt(out, out, tmp, Alu.add)

                def fold_and(tile_ap, nblk, target):
                    n = nblk
                    while n > 1:
                        h = n // 2
                        lo = _mk(tile_ap, 0, [[1, h * W]])
                        hi = _mk(tile_ap, (n - h) * W, [[1, h * W]])
                        tt(lo, lo, hi, Alu.min)
                        n = n - h
                    if target is not None:
                        tt(target, target, _mk(tile_ap, 0, [[1, W]]), Alu.min)

                # row edges, both sides
                E = {}
                for t in (0, 1):
                    for c in range(3):
                        e = T3(f"E{c}_{t}")
                        nxt = _mk(Gap, 9 * t + c + 3, [[3, 2], [18, W]])
                        cur = _mk(Gap, 9 * t + c, [[3, 2], [18, W]])
                        tt(_mk(e, 0, [[W, 2], [1, W]]), nxt, cur, Alu.subtract)
                        tt(blkof(e, 2), _mk(Gap, 9 * t + c, [[18, W]]),
                           _mk(Gap, 9 * t + c + 6, [[18, W]]), Alu.subtract)
                        E[(t, c)] = e

                t9a = T9("t9a"); t9b = T9("t9b")
                # line-line: n = d1 x d2 per (i,j)
                nn = [T9(f"nn{c}") for c in range(3)]
                for c in range(3):
                    c1, c2 = (c + 1) % 3, (c + 2) % 3
                    tt(t9a, repA(E[(0, c1)]), repB(E[(1, c2)]), Alu.mult)
                    tt(t9b, repA(E[(0, c2)]), repB(E[(1, c1)]), Alu.mult)
                    tt(nn[c], t9a, t9b, Alu.subtract)
                r = [T9(f"r{c}") for c in range(3)]
                for c in range(3):
                    tt(s9(r[c]), R9A(c), R9B(c), Alu.subtract)
                rn = T9("rn"); dot3(rn, nn, r, t9a)
                n2 = T9("n2"); norm2(n2, nn, t9a)
                rn2 = T9("rn2")
                S.activation(out=rn2, in_=rn, func=Act.Square)
                fl9 = T9("fl9")
                V.scalar_tensor_tensor(out=fl9, in0=n2, scalar=M2, in1=rn2,
                                       op0=Alu.mult, op1=Alu.is_lt)
                V.scalar_tensor_tensor(out=fl9, in0=n2, scalar=N2MIN, in1=fl9,
                                       op0=Alu.is_gt, op1=Alu.mult)
                prune = TW("prune")
                V.memset(prune, 1.0)
                fold_and(fl9, 9, prune)

                # plane bounds per tri side
                for tp, tt_ in ((0, 1), (1, 0)):
                    eC0 = []
                    eC1 = []
                    for c in range(3):
                        x = TW(f"p1e0{c}")
                        tt(x, Cv(tt_, c, 1), Cv(tt_, c, 0), Alu.subtract)
                        eC0.append(x)
                        y = TW(f"p1e1{c}")
                        tt(y, Cv(tt_, c, 2), Cv(tt_, c, 0), Alu.subtract)
                        eC1.append(y)
                    nC = []
                    tw = TW("p1tw")
                    for c in range(3):
                        c1, c2 = (c + 1) % 3, (c + 2) % 3
                        z = TW(f"p1n{c}")
                        tt(tw, eC0[c1], eC1[c2], Alu.mult)
                        tt(z, eC0[c2], eC1[c1], Alu.mult)
                        tt(z, tw, z, Alu.subtract)
                        nC.append(z)
                    n2C = TW("p1n2"); norm2(n2C, nC, tw)
                    w3 = [T3(f"p1w{c}") for c in range(3)]
                    for c in range(3):
                        tt(s3(w3[c]), Rblk(tp, c), Cb3(tt_, c, 0), Alu.subtract)
                    wn = T3("p1wn"); dot3(wn, [b3(x) for x in nC], w3, T3("p1t3"))
                    wn2 = T3("p1wn2")
                    S.activation(out=wn2, in_=wn, func=Act.Square)
                    fl3 = T3("p1fl3")
                    V.scalar_tensor_tensor(out=fl3, in0=b3(n2C), scalar=M2, in1=s3(wn2),
                                           op0=Alu.mult, op1=Alu.is_lt)
                    V.scalar_tensor_tensor(out=fl3, in0=b3(n2C), scalar=N2MIN, in1=s3(fl3),
                                           op0=Alu.is_gt, op1=Alu.mult)
                    fold_and(fl3, 3, prune)

                nc.sync.dma_start(out=flags_out[:, colbase:colbase + W], in_=prune)
                colbase += W
    nc.compile()
    return nc


def kernel(triangles, close_idxs):
    triangles = np.ascontiguousarray(np.asarray(triangles, dtype=np.float32))
    ci = np.asarray(close_idxs)
    Bv, Pv = ci.shape[0], ci.shape[1]
    tbl = triangles.reshape(Bv * F, 9)

    recv_raw = ci[..., 0].reshape(-1)
    valid = recv_raw >= 0
    valid_count = max(float(valid.sum()), 1.0)

    ci32 = np.maximum(ci.astype(np.int64), 0).astype(np.int32)
    flat = ci32.reshape(-1, 2)
    batch_off = (np.arange(NPAIR, dtype=np.int64) // PPB * F).astype(np.int32)
    flat_abs = flat + batch_off[:, None]

    trace = bool(os.environ.get("BASS_KERNEL_TRACE"))
    tkw = dict(trace=trace, trace_cores=[0] if trace else None)
    exec_ns = 0

    if not os.environ.get("K_TWO_PHASE"):
        if "nc" not in _CACHE:
            _CACHE["nc"] = _build_kernel()
        nc = _CACHE["nc"]
        in_maps = []
        for c in range(NCORE):
            sl = flat_abs[c * PER_CORE:(c + 1) * PER_CORE]
            grid = np.zeros((CAP, 2), np.int32)
            grid[:PER_CORE] = sl
            mask = np.zeros(CAP, np.float32)
            mask[:PER_CORE] = valid[c * PER_CORE:(c + 1) * PER_CORE]
            gd = tbl[grid.reshape(-1)].reshape(CAP, 18).reshape(P, 18 * NCOL)
            in_maps.append({"gdata": gd, "maskin": mask.reshape(P, NCOL)})
        res = run_bass_kernel_spmd(nc, in_maps, list(range(NCORE)), **tkw)
        _CACHE["exec_time_ns"] = res.exec_time_ns
        total = sum(float(res.results[c]["psum"].astype(np.float64).sum())
                    for c in range(NCORE))
        return np.asarray(np.float32(total / valid_count))

    # ---------------- phase 1: certified prune over all pairs ----------------
    if "nc_flags" not in _CACHE:
        _CACHE["nc_flags"] = _build_flags()
    ncf = _CACHE["nc_flags"]
    CAP1 = P * NCOL1
    in_maps = []
    for c in range(NCORE):
        grid = np.zeros((CAP1, 2), np.int32)
        grid[:PER_CORE] = flat_abs[c * PER_CORE:(c + 1) * PER_CORE]
        gd = tbl[grid.reshape(-1)].reshape(CAP1, 18).reshape(P, 18 * NCOL1)
        in_maps.append({"gdata": gd})
    res1 = run_bass_kernel_spmd(ncf, in_maps, list(range(NCORE)), **tkw)
    if res1.exec_time_ns:
        exec_ns += res1.exec_time_ns

    surv = []
    for c in range(NCORE):
        fl = res1.results[c]["flags"].reshape(-1)[:PER_CORE]
        loc = np.nonzero((fl < 0.5) & valid[c * PER_CORE:(c + 1) * PER_CORE])[0]
        surv.append(loc + c * PER_CORE)
    surv = np.concatenate(surv)
    _CACHE["n_survivors"] = int(surv.size)

    if surv.size == 0:
        _CACHE["exec_time_ns"] = exec_ns
        return np.asarray(np.float32(0.0))

    # ---------------- phase 2: exact evaluation of survivors ----------------
    if "nc_p2" not in _CACHE:
        _CACHE["nc_p2"] = _build_kernel(ncol=CAP2_COL, tile_w=[CAP2_COL])
    nc2 = _CACHE["nc_p2"]
    rows = flat_abs[surv]                      # [S, 2]
    total = 0.0
    chunk = CAP2 * NCORE
    for s0 in range(0, surv.size, chunk):
        sub = rows[s0:s0 + chunk]
        n = sub.shape[0]
        in_maps = []
        for c in range(NCORE):
            grid = np.zeros((CAP2, 2), np.int32)
            mask = np.zeros(CAP2, np.float32)
            lo, hi = c * CAP2, min((c + 1) * CAP2, n)
            if hi > lo:
                grid[:hi - lo] = sub[lo:hi]
                mask[:hi - lo] = 1.0
            gd = tbl[grid.reshape(-1)].reshape(CAP2, 18).reshape(P, 18 * CAP2_COL)
            in_maps.append({"gdata": gd, "maskin": mask.reshape(P, CAP2_COL)})
        res2 = run_bass_kernel_spmd(nc2, in_maps, list(range(NCORE)), **tkw)
        if res2.exec_time_ns:
            exec_ns += res2.exec_time_ns
        total += sum(float(res2.results[c]["psum"].astype(np.float64).sum())
                     for c in range(NCORE))

    _CACHE["exec_time_ns"] = exec_ns if exec_ns else None
    return np.asarray(np.float32(total / valid_count))



# revision 2
# speedup vs baseline: 1.0000x; 1.0000x over previous
"""Trainium2 Bass kernel for nn_DistanceFieldPenetrationLoss.

Computes loss = sum(relu(1e-3 - tridist(A,B))) / count over 2M close pairs,
sharded data-parallel over 8 NeuronCores. Per-pair triangle rows are
pre-gathered on the host (HW indirect-DMA gathers are one-index-per-
partition on TRN2, making on-device gathers descriptor-bound) and streamed
to SBUF as contiguous DMA; all geometry runs on-device in a term-blocked
SoA layout (strided/broadcast AP views over the gathered tile, 1 instr per
blocked group of the 15 distance terms).

The per-pair triangle "distance" replicates the reference exactly:
min over 15 terms: 6 point-(column-)triangle distances + 9 row-edge/edge
distances. Point-triangle = {face-masked, 3 point-edge}; edge-edge is
evaluated in its exact boundary form: min(interior-masked, 4 point-edge),
which equals the reference's clamp/recompute algorithm mathematically.
"""
import numpy as np

import concourse.bass as bass
import concourse.bacc as bacc
import concourse.mybir as mybir
import concourse.tile as tile
from concourse.bass_utils import run_bass_kernel_spmd

F32 = mybir.dt.float32
I32 = mybir.dt.int32
Alu = mybir.AluOpType
Act = mybir.ActivationFunctionType

P = 128
B, F, PPB = 4, 50000, 500000
NPAIR = B * PPB
NCORE = 8
PER_CORE = NPAIR // NCORE          # 250000
NCOL = 1954                        # 128*1954 = 250112 slots per core
CAP = P * NCOL
import os
if os.environ.get("K_DEBUG_SMALL"):
    NCOL = 8
    CAP = P * NCOL
    TILE_W = [8]
else:
    TILE_W = [152] * 12 + [130]        # sum = 1954
EPS = 1e-12
LOSS_EPS = 1e-3
BIG = 1e30

_CACHE = {}


def _mk(ap, off, dims):
    """View into an SBUF tile AP with explicit free dims [[step, count], ...]."""
    return bass.AP(ap.tensor, ap.offset + off, [list(ap.ap[0])] + [list(d) for d in dims])


def _build_kernel(ncol=None, tile_w=None):
    NCOL = ncol if ncol is not None else globals()["NCOL"]
    TILE_W = tile_w if tile_w is not None else globals()["TILE_W"]
    nc = bacc.Bacc("TRN2", target_bir_lowering=False, debug=False)
    gdata = nc.declare_dram_parameter("gdata", [P, 18 * NCOL], F32, isOutput=False)
    maskin = nc.declare_dram_parameter("maskin", [P, NCOL], F32, isOutput=False)
    psum_out = nc.declare_dram_parameter("psum", [P, len(TILE_W)], F32, isOutput=True)
    DEBUG = bool(os.environ.get("K_DEBUG_SMALL"))
    if DEBUG:
        dist_out = nc.declare_dram_parameter("dist", [P, NCOL], F32, isOutput=True)
        accpt_out = nc.declare_dram_parameter("accpt", [P, NCOL], F32, isOutput=True)

    with tile.TileContext(nc) as tc:
        with (
            tc.tile_pool(name="gio", bufs=2) as gio,
            tc.tile_pool(name="work", bufs=1) as work,
        ):
            V = nc.vector
            S = nc.scalar

            psum_t = work.tile([P, len(TILE_W)], F32, tag="psum", name="psum")
            V.memset(psum_t[:], 0.0)

            colbase = 0
            for ti, W in enumerate(TILE_W):
                G = gio.tile([P, 18 * W], F32, tag="g", name="g")
                nc.sync.dma_start(out=G[:], in_=gdata[:, 18 * colbase:18 * (colbase + W)])
                M = gio.tile([P, W], F32, tag="mask", name="mask")
                nc.sync.dma_start(out=M[:], in_=maskin[:, colbase:colbase + W])

                Gap = G[:]

                # --- G views.  free index = w*18 + t*9 + m;  m = 3*row + col(coord)
                # row-vertex i, comp c of side t:   m = 3i + c
                # col-vertex k, comp c of side t:   m = 3c + k
                def Rblk(t, c):          # [3(vert i), W]
                    return _mk(Gap, 9 * t + c, [[3, 3], [18, W]])

                def R9A(c):              # [3(i), 3(rep j), W]
                    return _mk(Gap, c, [[3, 3], [0, 3], [18, W]])

                def R9B(c):              # [3(rep i), 3(j), W]
                    return _mk(Gap, 9 + c, [[0, 3], [3, 3], [18, W]])

                def Cv(t, c, k):         # [W] single col-vertex comp
                    return _mk(Gap, 9 * t + 3 * c + k, [[18, W]])

                def Cb3(t, c, k):        # [3(rep), W]
                    return _mk(Gap, 9 * t + 3 * c + k, [[0, 3], [18, W]])

                # tile allocation helpers (plain + shaped views)
                def TW(tag):
                    return work.tile([P, W], F32, tag=tag, name=tag)[:]

                def T3(tag):
                    return work.tile([P, 3 * W], F32, tag=tag, name=tag)[:]

                def T9(tag):
                    return work.tile([P, 9 * W], F32, tag=tag, name=tag)[:]

                def s3(ap):              # [3, W] view of 3W tile
                    return _mk(ap, 0, [[W, 3], [1, W]])

                def s9(ap):              # [3, 3, W] view of 9W tile
                    return _mk(ap, 0, [[3 * W, 3], [W, 3], [1, W]])

                def b3(ap_w):            # broadcast [W] tile over 3 blocks
                    return _mk(ap_w, 0, [[0, 3], [1, W]])

                def repA(ap3):           # [3W] tile -> [3(i), 3(rep), W]
                    return _mk(ap3, 0, [[W, 3], [0, 3], [1, W]])

                def repB(ap3):           # [3W] tile -> [3(rep), 3(j), W]
                    return _mk(ap3, 0, [[0, 3], [W, 3], [1, W]])

                def blkof(ap3, i):       # i-th W block of a 3W tile
                    return _mk(ap3, i * W, [[1, W]])

                GP = nc.gpsimd

                def tt(out, a, b, op, eng=None):
                    (eng or V).tensor_tensor(out=out, in0=a, in1=b, op=op)

                def dot3g(out, av, bv, tmp):
                    GP.tensor_tensor(out=tmp, in0=av[0], in1=bv[0], op=Alu.mult)
                    GP.tensor_tensor(out=out, in0=av[1], in1=bv[1], op=Alu.mult)
                    GP.tensor_tensor(out=out, in0=out, in1=tmp, op=Alu.add)
                    GP.tensor_tensor(out=tmp, in0=av[2], in1=bv[2], op=Alu.mult)
                    GP.tensor_tensor(out=out, in0=out, in1=tmp, op=Alu.add)

                def dot3(out, av, bv, tmp):
                    tt(tmp, av[0], bv[0], Alu.mult)
                    tt(out, av[1], bv[1], Alu.mult)
                    tt(out, out, tmp, Alu.add)
                    tt(tmp, av[2], bv[2], Alu.mult)
                    tt(out, out, tmp, Alu.add)

                def norm2(out, av, tmp):
                    S.activation(out=tmp, in_=av[0], func=Act.Square)
                    S.activation(out=out, in_=av[1], func=Act.Square)
                    tt(out, out, tmp, Alu.add)
                    S.activation(out=tmp, in_=av[2], func=Act.Square)
                    tt(out, out, tmp, Alu.add)

                def recip(out, x, tmp):
                    S.activation(out=tmp, in_=x, func=Act.Ln)
                    S.activation(out=out, in_=tmp, func=Act.Exp, scale=-1.0)

                def clip01(x):
                    V.tensor_scalar(out=x, in0=x, scalar1=0.0, scalar2=1.0,
                                    op0=Alu.max, op1=Alu.min)

                acc = TW("acc")
                V.memset(acc, BIG)

                def foldmin(blocked_ap_tile, nblk):
                    # min of nblk W-blocks of a tile into acc
                    n = nblk
                    while n > 1:
                        h = n // 2
                        lo = _mk(blocked_ap_tile, 0, [[1, h * W]])
                        hi = _mk(blocked_ap_tile, (n - h) * W, [[1, h * W]])
                        tt(lo, lo, hi, Alu.min)
                        n = n - h
                    tt(acc, acc, _mk(blocked_ap_tile, 0, [[1, W]]), Alu.min)

                # ---------- per-side derived data ----------
                side = []
                for t in (0, 1):
                    sd = {}
                    # column-triangle data
                    eC = []
                    for pair_kk in ((1, 0), (2, 0), (2, 1)):
                        comps = []
                        for c in range(3):
                            e = TW(f"eC{len(eC)}{c}_{t}")
                            tt(e, Cv(t, c, pair_kk[0]), Cv(t, c, pair_kk[1]), Alu.subtract)
                            comps.append(e)
                        eC.append(comps)
                    sd["eC"] = eC
                    tmpw = TW(f"tmpw{t}")
                    aC = TW(f"aC{t}"); norm2(aC, eC[0], tmpw)
                    V.tensor_scalar(out=aC, in0=aC, scalar1=EPS, scalar2=None, op0=Alu.max)
                    bC = TW(f"bC{t}"); dot3(bC, eC[0], eC[1], tmpw)
                    cC = TW(f"cC{t}"); norm2(cC, eC[1], tmpw)
                    V.tensor_scalar(out=cC, in0=cC, scalar1=EPS, scalar2=None, op0=Alu.max)
                    a2C = TW(f"a2C{t}"); norm2(a2C, eC[2], tmpw)
                    V.tensor_scalar(out=a2C, in0=a2C, scalar1=EPS, scalar2=None, op0=Alu.max)
                    det = TW(f"det{t}")
                    S.activation(out=tmpw, in_=bC, func=Act.Square)
                    tt(det, aC, cC, Alu.mult)
                    tt(det, det, tmpw, Alu.subtract)
                    V.tensor_scalar(out=det, in0=det, scalar1=EPS, scalar2=None, op0=Alu.max)
                    for nm, src in (("invdet", det), ("invaC", aC), ("invcC", cC), ("inva2C", a2C)):
                        dst = TW(nm + str(t)); recip(dst, src, tmpw)
                        sd[nm] = dst
                    sd.update(aC=aC, bC=bC, cC=cC, a2C=a2C, det=det)

                    # row-edge data (3W blocked: edges [R1-R0, R2-R1, R0-R2])
                    E = []
                    for c in range(3):
                        e = T3(f"E{c}_{t}")
                        # blocks 0..1: R_{i+1} - R_i
                        nxt = _mk(Gap, 9 * t + c + 3, [[3, 2], [18, W]])
                        cur = _mk(Gap, 9 * t + c, [[3, 2], [18, W]])
                        tt(_mk(e, 0, [[W, 2], [1, W]]), nxt, cur, Alu.subtract)
                        # block 2: R0 - R2
                        tt(blkof(e, 2), _mk(Gap, 9 * t + c, [[18, W]]),
                           _mk(Gap, 9 * t + c + 6, [[18, W]]), Alu.subtract)
                        E.append(e)
                    sd["E"] = E
                    tmp3 = T3(f"tmp3_{t}")
                    aE = T3(f"aE{t}")
                    norm2(aE, E, tmp3)
                    invE = T3(f"invE{t}"); recip(invE, aE, tmp3)
                    ninvE = T3(f"ninvE{t}")
                    V.tensor_scalar(out=ninvE, in0=invE, scalar1=-1.0, scalar2=None, op0=Alu.mult)
                    sd.update(aE=aE, invE=invE, ninvE=ninvE)
                    side.append(sd)

                # ---------- point-triangle, 2 directions x 3 points (3W blocked) ----------
                t3a = T3("t3a"); t3b = T3("t3b"); t3c = T3("t3c")
                for tp, tt_ in ((0, 1), (1, 0)):
                    sd = side[tt_]
                    w = [T3(f"w{c}") for c in range(3)]
                    for c in range(3):
                        tt(s3(w[c]), Rblk(tp, c), Cb3(tt_, c, 0), Alu.subtract)
                    d = T3("ptd"); dot3(d, [b3(x) for x in sd["eC"][0]], w, t3a)
                    e = T3("pte"); dot3(e, [b3(x) for x in sd["eC"][1]], w, t3a)
                    f = T3("ptf"); norm2(f, w, t3a)
                    s = T3("pts")
                    tt(t3a, b3(sd["bC"]), e, Alu.mult)
                    tt(t3b, b3(sd["cC"]), d, Alu.mult)
                    tt(s, t3a, t3b, Alu.subtract)
                    t = T3("ptt")
                    tt(t3a, b3(sd["bC"]), d, Alu.mult)
                    tt(t3b, b3(sd["aC"]), e, Alu.mult)
                    tt(t, t3a, t3b, Alu.subtract)
                    # in-face margin m = min(s, t, det-(s+t))
                    m = T3("ptm")
                    tt(m, s, t, Alu.min)
                    tt(t3a, s, t, Alu.add)
                    V.scalar_tensor_tensor(out=t3b, in0=t3a, scalar=-1.0, in1=b3(sd["det"]),
                                           op0=Alu.mult, op1=Alu.add)
                    tt(m, m, t3b, Alu.min)
                    # face distance
                    fc = T3("ptfc")
                    tt(t3a, d, s, Alu.mult)
                    tt(t3b, e, t, Alu.mult)
                    tt(t3a, t3a, t3b, Alu.add)
                    tt(t3b, f, b3(sd["det"]), Alu.mult)
                    tt(t3a, t3b, t3a, Alu.subtract)
                    tt(fc, t3a, b3(sd["invdet"]), Alu.mult)
                    V.tensor_scalar(out=fc, in0=fc, scalar1=0.0, scalar2=None, op0=Alu.max)
                    V.tensor_scalar(out=t3a, in0=m, scalar1=0.0, scalar2=BIG,
                                    op0=Alu.is_lt, op1=Alu.mult)
                    tt(fc, fc, t3a, Alu.add)
                    foldmin(fc, 3)
                    # pe01 / pe02: foot on eC0 (param d/aC) and eC1 (param e/cC)
                    for dotv, inv, ev in ((d, "invaC", 0), (e, "invcC", 1)):
                        u = t3c
                        tt(u, dotv, b3(sd[inv]), Alu.mult)
                        clip01(u)
                        pe = T3("ptpe")
                        for c in range(3):
                            tt(t3a, u, b3(sd["eC"][ev][c]), Alu.mult)
                            tt(w2c := t3b, w[c], t3a, Alu.subtract)
                            if c == 0:
                                S.activation(out=pe, in_=w2c, func=Act.Square)
                            else:
                                S.activation(out=t3a, in_=w2c, func=Act.Square)
                                tt(pe, pe, t3a, Alu.add)
                        foldmin(pe, 3)
                    # pe12: w2 = p - C1, edge eC2, param dd/a2C
                    w2 = [T3(f"w2{c}") for c in range(3)]
                    for c in range(3):
                        tt(s3(w2[c]), Rblk(tp, c), Cb3(tt_, c, 1), Alu.subtract)
                    dd = T3("ptdd"); dot3(dd, [b3(x) for x in sd["eC"][2]], w2, t3a)
                    u = t3c
                    tt(u, dd, b3(sd["inva2C"]), Alu.mult)
                    clip01(u)
                    pe = T3("ptpe")
                    for c in range(3):
                        tt(t3a, u, b3(sd["eC"][2][c]), Alu.mult)
                        tt(t3b, w2[c], t3a, Alu.subtract)
                        if c == 0:
                            S.activation(out=pe, in_=t3b, func=Act.Square)
                        else:
                            S.activation(out=t3a, in_=t3b, func=Act.Square)
                            tt(pe, pe, t3a, Alu.add)
                    foldmin(pe, 3)

                if DEBUG:
                    nc.sync.dma_start(out=accpt_out[:, colbase:colbase + W], in_=acc)
                # ---------- edge-edge, 9-blocked [3(i:A-edge), 3(j:B-edge), W] ----------
                EA, EB = side[0]["E"], side[1]["E"]
                t9a = T9("t9a"); t9b = T9("t9b"); t9c = T9("t9c")
                r = [T9(f"r{c}") for c in range(3)]
                for c in range(3):
                    tt(s9(r[c]), R9A(c), R9B(c), Alu.subtract)
                d1v = [repA(EA[c]) for c in range(3)]
                d2v = [repB(EB[c]) for c in range(3)]
                cdot = T9("cdot"); dot3(cdot, d1v, r, t9a)
                fdot = T9("fdot"); dot3(fdot, d2v, r, t9a)
                bq = T9("bq"); dot3(bq, d1v, d2v, t9a)
                aA_r = repA(side[0]["aE"]); aB_t = repB(side[1]["aE"])
                den = T9("den")
                tt(den, aA_r, aB_t, Alu.mult)
                S.activation(out=t9a, in_=bq, func=Act.Square)
                tt(den, den, t9a, Alu.subtract)
                dadj = T9("dadj")
                V.tensor_scalar(out=dadj, in0=den, scalar1=EPS, scalar2=None, op0=Alu.max)
                invd = T9("invd")
                S.activation(out=t9a, in_=dadj, func=Act.Ln)
                S.activation(out=invd, in_=t9a, func=Act.Exp, scale=-1.0)
                s = T9("ees")
                tt(t9a, bq, fdot, Alu.mult)
                tt(t9b, cdot, aB_t, Alu.mult)
                tt(s, t9a, t9b, Alu.subtract)
                tt(s, s, invd, Alu.mult)
                t = T9("eet")
                tt(t9a, aA_r, fdot, Alu.mult)
                tt(t9b, bq, cdot, Alu.mult)
                tt(t, t9a, t9b, Alu.subtract)
                tt(t, t, invd, Alu.mult)
                # Clamp s,t to [0,1]: identical when the interior solution is
                # valid; otherwise the clamped point-pair distance upper-bounds
                # the true segment distance, which the boundary point-edge
                # terms below already realize — the 15-term min is unchanged.
                clip01(s)
                clip01(t)
                d2i = T9("d2i")
                for c in range(3):
                    tt(t9a, s, d1v[c], Alu.mult)
                    tt(t9a, r[c], t9a, Alu.add)
                    tt(t9b, t, d2v[c], Alu.mult)
                    tt(t9a, t9a, t9b, Alu.subtract)
                    if c == 0:
                        S.activation(out=d2i, in_=t9a, func=Act.Square)
                    else:
                        S.activation(out=t9b, in_=t9a, func=Act.Square)
                        tt(d2i, d2i, t9b, Alu.add)
                foldmin(d2i, 9)
                # pe(A_i, B-edge j): u = clip(fdot * invE_B); v = r - u*d2
                u = t9c
                tt(u, fdot, repB(side[1]["invE"]), Alu.mult)
                clip01(u)
                pe9 = T9("pe9")
                for c in range(3):
                    tt(t9a, u, d2v[c], Alu.mult)
                    tt(t9a, r[c], t9a, Alu.subtract)
                    if c == 0:
                        S.activation(out=pe9, in_=t9a, func=Act.Square)
                    else:
                        S.activation(out=t9b, in_=t9a, func=Act.Square)
                        tt(pe9, pe9, t9b, Alu.add)
                foldmin(pe9, 9)
                # pe(B_j, A-edge i): u = clip(cdot * -invE_A); v = r + u*d1
                tt(u, cdot, repA(side[0]["ninvE"]), Alu.mult)
                clip01(u)
                for c in range(3):
                    tt(t9a, u, d1v[c], Alu.mult)
                    tt(t9a, r[c], t9a, Alu.add)
                    if c == 0:
                        S.activation(out=pe9, in_=t9a, func=Act.Square)
                    else:
                        S.activation(out=t9b, in_=t9a, func=Act.Square)
                        tt(pe9, pe9, t9b, Alu.add)
                foldmin(pe9, 9)

                # ---------- pen = relu(1e-3 - sqrt(acc)); masked accumulate ----------
                dist = TW("dist")
                S.activation(out=dist, in_=acc, func=Act.Sqrt)
                pen = TW("pen")
                V.tensor_scalar(out=pen, in0=dist, scalar1=-1.0, scalar2=LOSS_EPS,
                                op0=Alu.mult, op1=Alu.add)
                if DEBUG:
                    nc.sync.dma_start(out=dist_out[:, colbase:colbase + W], in_=dist)
                penm = TW("penm")
                V.scalar_tensor_tensor(out=penm, in0=pen, scalar=0.0, in1=M[:],
                                       op0=Alu.max, op1=Alu.mult,
                                       accum_out=psum_t[:, ti:ti + 1])
                colbase += W

            nc.sync.dma_start(out=psum_out[:], in_=psum_t[:])
    nc.compile()
    return nc




M2 = 4e-6        # (2e-3)^2 certified prune margin squared
N2MIN = 1e-4     # min |cross|^2 for a trustworthy normal direction

NCOL1 = 1954
TILE_W1 = [128] * 15 + [34]
CAP2_COL = 128                     # phase-2 slots per core = 128*128 = 16384
CAP2 = P * CAP2_COL


def _build_flags():
    """Phase-1: certified lower-bound prune. Writes per-slot flag:
    1.0 = certifiably all 15 terms > 1e-3 (pen == 0), 0.0 = needs phase 2."""
    NCOL = NCOL1
    TILE_W = TILE_W1
    nc = bacc.Bacc("TRN2", target_bir_lowering=False, debug=False)
    gdata = nc.declare_dram_parameter("gdata", [P, 18 * NCOL], F32, isOutput=False)
    flags_out = nc.declare_dram_parameter("flags", [P, NCOL], F32, isOutput=True)

    with tile.TileContext(nc) as tc:
        with (
            tc.tile_pool(name="gio", bufs=2) as gio,
            tc.tile_pool(name="work", bufs=1) as work,
        ):
            V = nc.vector
            S = nc.scalar
            colbase = 0
            for ti, W in enumerate(TILE_W):
                G = gio.tile([P, 18 * W], F32, tag="g", name="g")
                nc.sync.dma_start(out=G[:], in_=gdata[:, 18 * colbase:18 * (colbase + W)])
                Gap = G[:]

                def Rblk(t, c):
                    return _mk(Gap, 9 * t + c, [[3, 3], [18, W]])

                def R9A(c):
                    return _mk(Gap, c, [[3, 3], [0, 3], [18, W]])

                def R9B(c):
                    return _mk(Gap, 9 + c, [[0, 3], [3, 3], [18, W]])

                def Cv(t, c, k):
                    return _mk(Gap, 9 * t + 3 * c + k, [[18, W]])

                def Cb3(t, c, k):
                    return _mk(Gap, 9 * t + 3 * c + k, [[0, 3], [18, W]])

                def TW(tag):
                    return work.tile([P, W], F32, tag=tag, name=tag)[:]

                def T3(tag):
                    return work.tile([P, 3 * W], F32, tag=tag, name=tag)[:]

                def T9(tag):
                    return work.tile([P, 9 * W], F32, tag=tag, name=tag)[:]

                def s3(ap):
                    return _mk(ap, 0, [[W, 3], [1, W]])

                def s9(ap):
                    return _mk(ap, 0, [[3 * W, 3], [W, 3], [1, W]])

                def b3(ap_w):
                    return _mk(ap_w, 0, [[0, 3], [1, W]])

                def repA(ap3):
                    return _mk(ap3, 0, [[W, 3], [0, 3], [1, W]])

                def repB(ap3):
                    return _mk(ap3, 0, [[0, 3], [W, 3], [1, W]])

                def blkof(ap3, i):
                    return _mk(ap3, i * W, [[1, W]])

                def tt(out, a, b, op):
                    V.tensor_tensor(out=out, in0=a, in1=b, op=op)

                def dot3(out, av, bv, tmp):
                    tt(tmp, av[0], bv[0], Alu.mult)
                    tt(out, av[1], bv[1], Alu.mult)
                    tt(out, out, tmp, Alu.add)
                    tt(tmp, av[2], bv[2], Alu.mult)
                    tt(out, out, tmp, Alu.add)

                def norm2(out, av, tmp):
                    S.activation(out=tmp, in_=av[0], func=Act.Square)
                    S.activation(out=out, in_=av[1], func=Act.Square)
                    tt(out, out, tmp, Alu.add)
                    S.activation(out=tmp, in_=av[2], func=Act.Square)
                    tt(out, out, tmp, Alu.add)

                def fold_and(tile_ap, nblk, target):
                    n = nblk
                    while n > 1:
                        h = n // 2
                        lo = _mk(tile_ap, 0, [[1, h * W]])
                        hi = _mk(tile_ap, (n - h) * W, [[1, h * W]])
                        tt(lo, lo, hi, Alu.min)
                        n = n - h
                    if target is not None:
                        tt(target, target, _mk(tile_ap, 0, [[1, W]]), Alu.min)

                # row edges, both sides
                E = {}
                for t in (0, 1):
                    for c in range(3):
                        e = T3(f"E{c}_{t}")
                        nxt = _mk(Gap, 9 * t + c + 3, [[3, 2], [18, W]])
                        cur = _mk(Gap, 9 * t + c, [[3, 2], [18, W]])
                        tt(_mk(e, 0, [[W, 2], [1, W]]), nxt, cur, Alu.subtract)
                        tt(blkof(e, 2), _mk(Gap, 9 * t + c, [[18, W]]),
                           _mk(Gap, 9 * t + c + 6, [[18, W]]), Alu.subtract)
                        E[(t, c)] = e

                t9a = T9("t9a"); t9b = T9("t9b")
                # line-line: n = d1 x d2 per (i,j)
                nn = [T9(f"nn{c}") for c in range(3)]
                for c in range(3):
                    c1, c2 = (c + 1) % 3, (c + 2) % 3
                    tt(t9a, repA(E[(0, c1)]), repB(E[(1, c2)]), Alu.mult)
                    tt(t9b, repA(E[(0, c2)]), repB(E[(1, c1)]), Alu.mult)
                    tt(nn[c], t9a, t9b, Alu.subtract)
                r = [T9(f"r{c}") for c in range(3)]
                for c in range(3):
                    tt(s9(r[c]), R9A(c), R9B(c), Alu.subtract)
                rn = T9("rn"); dot3(rn, nn, r, t9a)
                n2 = T9("n2"); norm2(n2, nn, t9a)
                rn2 = T9("rn2")
                S.activation(out=rn2, in_=rn, func=Act.Square)
                fl9 = T9("fl9")
                V.scalar_tensor_tensor(out=fl9, in0=n2, scalar=M2, in1=rn2,
                                       op0=Alu.mult, op1=Alu.is_lt)
                V.scalar_tensor_tensor(out=fl9, in0=n2, scalar=N2MIN, in1=fl9,
                                       op0=Alu.is_gt, op1=Alu.mult)
                prune = TW("prune")
                V.memset(prune, 1.0)
                fold_and(fl9, 9, prune)

                # plane bounds per tri side
                for tp, tt_ in ((0, 1), (1, 0)):
                    eC0 = []
                    eC1 = []
                    for c in range(3):
                        x = TW(f"p1e0{c}")
                        tt(x, Cv(tt_, c, 1), Cv(tt_, c, 0), Alu.subtract)
                        eC0.append(x)
                        y = TW(f"p1e1{c}")
                        tt(y, Cv(tt_, c, 2), Cv(tt_, c, 0), Alu.subtract)
                        eC1.append(y)
                    nC = []
                    tw = TW("p1tw")
                    for c in range(3):
                        c1, c2 = (c + 1) % 3, (c + 2) % 3
                        z = TW(f"p1n{c}")
                        tt(tw, eC0[c1], eC1[c2], Alu.mult)
                        tt(z, eC0[c2], eC1[c1], Alu.mult)
                        tt(z, tw, z, Alu.subtract)
                        nC.append(z)
                    n2C = TW("p1n2"); norm2(n2C, nC, tw)
                    w3 = [T3(f"p1w{c}") for c in range(3)]
                    for c in range(3):
                        tt(s3(w3[c]), Rblk(tp, c), Cb3(tt_, c, 0), Alu.subtract)
                    wn = T3("p1wn"); dot3(wn, [b3(x) for x in nC], w3, T3("p1t3"))
                    wn2 = T3("p1wn2")
                    S.activation(out=wn2, in_=wn, func=Act.Square)
                    fl3 = T3("p1fl3")
                    V.scalar_tensor_tensor(out=fl3, in0=b3(n2C), scalar=M2, in1=s3(wn2),
                                           op0=Alu.mult, op1=Alu.is_lt)
                    V.scalar_tensor_tensor(out=fl3, in0=b3(n2C), scalar=N2MIN, in1=s3(fl3),
                                           op0=Alu.is_gt, op1=Alu.mult)
                    fold_and(fl3, 3, prune)

                nc.sync.dma_start(out=flags_out[:, colbase:colbase + W], in_=prune)
                colbase += W
    nc.compile()
    return nc


def kernel(triangles, close_idxs):
    triangles = np.ascontiguousarray(np.asarray(triangles, dtype=np.float32))
    ci = np.asarray(close_idxs)
    Bv, Pv = ci.shape[0], ci.shape[1]
    tbl = triangles.reshape(Bv * F, 9)

    recv_raw = ci[..., 0].reshape(-1)
    valid = recv_raw >= 0
    valid_count = max(float(valid.sum()), 1.0)

    ci32 = np.maximum(ci.astype(np.int64), 0).astype(np.int32)
    flat = ci32.reshape(-1, 2)
    batch_off = (np.arange(NPAIR, dtype=np.int64) // PPB * F).astype(np.int32)
    flat_abs = flat + batch_off[:, None]

    trace = bool(os.environ.get("BASS_KERNEL_TRACE"))
    tkw = dict(trace=trace, trace_cores=[0] if trace else None)
    exec_ns = 0

    if not os.environ.get("K_TWO_PHASE"):
        if "nc" not in _CACHE:
            _CACHE["nc"] = _build_kernel()
        nc = _CACHE["nc"]
        in_maps = []
        for c in range(NCORE):
            sl = flat_abs[c * PER_CORE:(c + 1) * PER_CORE]
            grid = np.zeros((CAP, 2), np.int32)
            grid[:PER_CORE] = sl
            mask = np.zeros(CAP, np.float32)
            mask[:PER_CORE] = valid[c * PER_CORE:(c + 1) * PER_CORE]
            gd = tbl[grid.reshape(-1)].reshape(CAP, 18).reshape(P, 18 * NCOL)
            in_maps.append({"gdata": gd, "maskin": mask.reshape(P, NCOL)})
        res = run_bass_kernel_spmd(nc, in_maps, list(range(NCORE)), **tkw)
        _CACHE["exec_time_ns"] = res.exec_time_ns
        if res.instructions_and_trace:
            _CACHE["trace_info"] = res.instructions_and_trace[1]
        total = sum(float(res.results[c]["psum"].astype(np.float64).sum())
                    for c in range(NCORE))
        return np.asarray(np.float32(total / valid_count))

    # ---------------- phase 1: certified prune over all pairs ----------------
    if "nc_flags" not in _CACHE:
        _CACHE["nc_flags"] = _build_flags()
    ncf = _CACHE["nc_flags"]
    CAP1 = P * NCOL1
    in_maps = []
    for c in range(NCORE):
        grid = np.zeros((CAP1, 2), np.int32)
        grid[:PER_CORE] = flat_abs[c * PER_CORE:(c + 1) * PER_CORE]
        gd = tbl[grid.reshape(-1)].reshape(CAP1, 18).reshape(P, 18 * NCOL1)
        in_maps.append({"gdata": gd})
    res1 = run_bass_kernel_spmd(ncf, in_maps, list(range(NCORE)), **tkw)
    if res1.exec_time_ns:
        exec_ns += res1.exec_time_ns

    surv = []
    for c in range(NCORE):
        fl = res1.results[c]["flags"].reshape(-1)[:PER_CORE]
        loc = np.nonzero((fl < 0.5) & valid[c * PER_CORE:(c + 1) * PER_CORE])[0]
        surv.append(loc + c * PER_CORE)
    surv = np.concatenate(surv)
    _CACHE["n_survivors"] = int(surv.size)

    if surv.size == 0:
        _CACHE["exec_time_ns"] = exec_ns
        return np.asarray(np.float32(0.0))

    # ---------------- phase 2: exact evaluation of survivors ----------------
    if "nc_p2" not in _CACHE:
        _CACHE["nc_p2"] = _build_kernel(ncol=CAP2_COL, tile_w=[CAP2_COL])
    nc2 = _CACHE["nc_p2"]
    rows = flat_abs[surv]                      # [S, 2]
    total = 0.0
    chunk = CAP2 * NCORE
    for s0 in range(0, surv.size, chunk):
        sub = rows[s0:s0 + chunk]
        n = sub.shape[0]
        in_maps = []
        for c in range(NCORE):
            grid = np.zeros((CAP2, 2), np.int32)
            mask = np.zeros(CAP2, np.float32)
            lo, hi = c * CAP2, min((c + 1) * CAP2, n)
            if hi > lo:
                grid[:hi - lo] = sub[lo:hi]
                mask[:hi - lo] = 1.0
            gd = tbl[grid.reshape(-1)].reshape(CAP2, 18).reshape(P, 18 * CAP2_COL)
            in_maps.append({"gdata": gd, "maskin": mask.reshape(P, CAP2_COL)})
        res2 = run_bass_kernel_spmd(nc2, in_maps, list(range(NCORE)), **tkw)
        if res2.exec_time_ns:
            exec_ns += res2.exec_time_ns
        total += sum(float(res2.results[c]["psum"].astype(np.float64).sum())
                     for c in range(NCORE))

    _CACHE["exec_time_ns"] = exec_ns if exec_ns else None
    return np.asarray(np.float32(total / valid_count))

